# revision 17
# baseline (speedup 1.0000x reference)
"""BatchHardTripletLoss on 8 Trainium2 NeuronCores — flipped + norm-dealt.

Layout: rows label-sorted on host; each core streams its 1024 anchors
(free dim) against all B=8192 embeddings as 64 column chunks of 128
(partition dim).

  - Own chunks (K, exact): the ~10 chunks holding the core's own labels
    (plus fillers), processed as K/2 psum pairs. PE adds a one-hot penalty
    matmul (+1024 on same-label); ACT copies psum->f16 with the exact
    per-partition sq_j bias fused (Identity + [128,1] bias AP). The
    hardest-pos partition-max runs per pair on the otherwise-idle GPSIMD
    (partition_all_reduce(max) -> one row DMA'd per pair; host maxes the
    rows). The hardest-neg side folds pairwise on DVE into a mini-tree.
  - Dealt chunks (64-K, approx): remaining columns are sorted by ||x||^2
    and dealt so each partition holds consecutive-rank norms across all
    chunks -> sq_j is near-constant per partition. Chunk pairs share one
    4-bank psum tile; most pairs go ACT Copy + f16 same-tile fold, some
    go via a strided DVE pair-min reduce straight from psum. A binary
    fold tree (f16 2x) collapses everything; the per-partition bias
    s_hat is fused into the final merge STT. The last pair takes the
    strided path and merges post-collapse so the tail stays short.
  - hardest-neg partition-min via PE transposes + DVE free-dim reduces;
    sqrt/relu/mean finalize on host.
"""

import sys

import numpy as np

if "/opt/trn_rl_repo" not in sys.path:
    sys.path.insert(0, "/opt/trn_rl_repo")

from concourse import bacc, bass, bass_isa, mybir, tile
from concourse.bass_utils import run_bass_kernel_spmd

B = 8192
D = 128
C = 128
N_CORES = 8
R = B // N_CORES          # anchors per core
NCH = B // 128            # column chunks (64)
RT = R // 128             # anchor blocks for the tail transposes (8)
PEN = 1024.0

F16 = mybir.dt.float16
F32 = mybir.dt.float32
ALU = mybir.AluOpType
ACTF = mybir.ActivationFunctionType

_NC_CACHE = {}


def _build_nc(kown):
    npair = (NCH - kown) // 2
    nown = kown // 2
    dve_pairs = {4, 9, 14, 19, 24, npair - 1}
    nc = bacc.Bacc(None, target_bir_lowering=False)

    xt_d = nc.declare_dram_parameter("xt", [128, B], F16, isOutput=False)
    xls_d = nc.declare_dram_parameter("xls", [128, R], F16, isOutput=False)
    ohs_d = nc.declare_dram_parameter("ohs", [128, R], F16, isOutput=False)
    ohk_d = nc.declare_dram_parameter("ohk", [128, kown * 128], F16, isOutput=False)
    sqc_d = nc.declare_dram_parameter("sqc", [128, kown], F32, isOutput=False)
    shat_d = nc.declare_dram_parameter("shat", [128, 1], F32, isOutput=False)
    idn_d = nc.declare_dram_parameter("idn", [128, 128], F16, isOutput=False)
    hn2_d = nc.declare_dram_parameter("hn2", [128, RT], F32, isOutput=True)
    hpr_d = nc.declare_dram_parameter("hpr", [1, nown * 2048], F32, isOutput=True)

    with tile.TileContext(nc) as tc:
        with tc.tile_pool(name="const", bufs=1) as cp:
            XTS = [cp.tile([128, 1024], F16, name=f"xts{s}") for s in range(8)]
            XLS = cp.tile([128, R], F16)
            OHS = cp.tile([128, R], F16)
            OHK = cp.tile([128, kown * 128], F16)
            SQC = cp.tile([128, kown], F32)
            SHAT = cp.tile([128, 1], F32)
            IDN = cp.tile([128, 128], F16)
            ACCF = cp.tile([128, R], F16)
            HN2 = cp.tile([128, RT], F32)

            # parallel queues: scalar issues the stream operand while sync
            # starts on the first dealt slice
            nc.scalar.dma_start(XLS[:], xls_d[:])
            nc.scalar.dma_start(SQC[:], sqc_d[:])
            nc.sync.dma_start(XTS[1][:], xt_d[:, 1024:2048])
            nc.sync.dma_start(XTS[2][:], xt_d[:, 2048:3072])
            nc.sync.dma_start(OHK[:], ohk_d[:])
            nc.sync.dma_start(OHS[:], ohs_d[:])
            nc.sync.dma_start(XTS[0][:], xt_d[:, 0:1024])
            for s in range(3, 8):
                nc.sync.dma_start(XTS[s][:], xt_d[:, s * 1024 : (s + 1) * 1024])
            nc.sync.dma_start(SHAT[:], shat_d[:])
            nc.sync.dma_start(IDN[:], idn_d[:])

            def chunk_lhs(ch):
                return XTS[ch // 8][:, (ch % 8) * 128 : (ch % 8) * 128 + 128]

            def make_tree():
                levels = {}

                def push(level, write_fn, tp, prefix):
                    buf = levels.get(level)
                    if buf is None:
                        nb = tp.tile([128, 2048], F16, tag=f"{prefix}{level}")
                        write_fn(nb[:, 0:1024])
                        levels[level] = nb
                    else:
                        write_fn(buf[:, 1024:2048])
                        levels[level] = None
                        push(level + 1,
                             lambda dst, b=buf: nc.vector.tensor_tensor(
                                 dst, b[:, 0:1024], b[:, 1024:2048], op=ALU.min),
                             tp, prefix)

                def collapse(tp, prefix):
                    pend = [levels[lv][:, 0:1024]
                            for lv in sorted(levels) if levels[lv] is not None]
                    nx = 0
                    while len(pend) > 1:
                        a = pend.pop(0)
                        b = pend.pop(0)
                        nb = tp.tile([128, 1024], F16, tag=f"{prefix}x{nx}")
                        nx += 1
                        nc.vector.tensor_tensor(nb[:], a, b, op=ALU.min)
                        pend.append(nb[:])
                    return pend[0]

                return push, collapse

            d_push, d_collapse = make_tree()
            o_push, o_collapse = make_tree()

            # schedule: 2 dealt pairs warm up (only need XLS+XT1), then the
            # own pairs (their one-hot/sq DMAs have landed), then the rest;
            # the last dealt pair is handled after the tree collapse.
            sched = [("d", 0), ("d", 1)]
            sched += [("o", u) for u in range(nown)]
            sched += [("d", t) for t in range(2, npair - 1)]

            with (
                tc.tile_pool(name="dpsum", bufs=2, space=bass.MemorySpace.PSUM) as pd,
                tc.tile_pool(name="opool", bufs=5) as op,
                tc.tile_pool(name="ppool", bufs=2) as pp,
                tc.tile_pool(name="gpool", bufs=4) as gp,
                tc.tile_pool(name="tpool", bufs=2) as tp,
            ):
                def emit_pair(kind, t):
                    ps = pd.tile([128, 2048], F32, tag="ps")
                    for j in range(2):
                        c = 2 * t + j
                        lhs = chunk_lhs(c if kind == "o" else kown + 2 * t + j)
                        for h in range(2):
                            sl = slice(j * 1024 + h * 512, j * 1024 + (h + 1) * 512)
                            nc.tensor.matmul(ps[:, sl], lhs,
                                             XLS[:, h * 512 : (h + 1) * 512],
                                             start=True, stop=kind == "d")
                            if kind == "o":
                                nc.tensor.matmul(ps[:, sl],
                                                 OHK[:, c * 128 : (c + 1) * 128],
                                                 OHS[:, h * 512 : (h + 1) * 512],
                                                 start=False, stop=True)
                    if kind == "o":
                        T2 = op.tile([128, 2048], F16, tag="t2")
                        for j in range(2):
                            c = 2 * t + j
                            nc.scalar.activation(
                                T2[:, j * 1024 : (j + 1) * 1024],
                                ps[:, j * 1024 : (j + 1) * 1024],
                                ACTF.Identity, bias=SQC[:, c : c + 1])
                        # hardest-pos: per-pair partition max on idle gpsimd
                        PM = pp.tile([128, 2048], F32, tag="pm")
                        nc.gpsimd.partition_all_reduce(
                            PM[:], T2[:], channels=128,
                            reduce_op=bass_isa.ReduceOp.max)
                        nc.sync.dma_start(
                            hpr_d[:, t * 2048 : (t + 1) * 2048], PM[0:1, :])
                        # hardest-neg: same-tile pair fold into the own tree
                        o_push(0, lambda dst, g=T2: nc.vector.tensor_tensor(
                            dst, g[:, 0:1024], g[:, 1024:2048], op=ALU.min),
                            tp, "ot")
                    elif t in dve_pairs:
                        d_push(0, lambda dst, p=ps: nc.vector.tensor_reduce(
                            dst, p[:].rearrange("p (c i) -> p i c", c=2),
                            axis=mybir.AxisListType.X, op=ALU.min), tp, "dt")
                    else:
                        G = gp.tile([128, 2048], F16, tag="g")
                        nc.scalar.activation(G[:], ps[:], ACTF.Copy)
                        d_push(0, lambda dst, g=G: nc.vector.tensor_tensor(
                            dst, g[:, 0:1024], g[:, 1024:2048], op=ALU.min),
                            tp, "dt")

                for kind, t in sched:
                    emit_pair(kind, t)
                OM = o_collapse(tp, "ot")
                DC = d_collapse(tp, "dt")
                # last dealt pair: strided reduce, merged post-collapse so the
                # tail after the final matmul stays short
                ps = pd.tile([128, 2048], F32, tag="ps")
                for j in range(2):
                    lhs = chunk_lhs(kown + 2 * (npair - 1) + j)
                    for h in range(2):
                        sl = slice(j * 1024 + h * 512, j * 1024 + (h + 1) * 512)
                        nc.tensor.matmul(ps[:, sl], lhs,
                                         XLS[:, h * 512 : (h + 1) * 512],
                                         start=True, stop=True)
                WL = tp.tile([128, 1024], F16, tag="wl")
                nc.vector.tensor_reduce(
                    WL[:], ps[:].rearrange("p (c i) -> p i c", c=2),
                    axis=mybir.AxisListType.X, op=ALU.min)
                FD = tp.tile([128, 1024], F16, tag="fd")
                nc.vector.tensor_tensor(FD[:], DC, WL[:], op=ALU.min)
                nc.vector.scalar_tensor_tensor(
                    ACCF[:], FD[:], SHAT[:, 0:1], OM,
                    op0=ALU.add, op1=ALU.min,
                )

            with tc.tile_pool(name="fpsum", bufs=4, space=bass.MemorySpace.PSUM) as pf:
                for t in range(RT):
                    pn = pf.tile([128, 128], F16, tag="pn")
                    nc.tensor.transpose(pn[:], ACCF[:, t * 128 : (t + 1) * 128], IDN[:])
                    nc.vector.tensor_reduce(HN2[:, t : t + 1], pn[:],
                                            axis=mybir.AxisListType.X, op=ALU.min)

            nc.sync.dma_start(hn2_d[:], HN2[:])

    nc.compile()
    return nc


def _get_nc(kown):
    if kown not in _NC_CACHE:
        _NC_CACHE[kown] = _build_nc(kown)
    return _NC_CACHE[kown]


def _prep_in_maps(embeddings, labels):
    x = np.asarray(embeddings, dtype=np.float32)
    lab = np.asarray(labels).astype(np.int64)
    order = np.argsort(lab, kind="stable")
    lab_s = lab[order]
    xs = x[order]
    xt = np.ascontiguousarray(xs.T).astype(np.float16)   # [128, B]
    sq = (xs.astype(np.float64) ** 2).sum(1).astype(np.float32)
    idn = np.eye(128, dtype=np.float16)
    own_sets = []
    K = 0
    for m in range(N_CORES):
        mylab = lab_s[m * R : (m + 1) * R]
        own_idx = np.flatnonzero((lab_s >= mylab.min()) & (lab_s <= mylab.max()))
        own_sets.append(own_idx)
        K = max(K, -(-len(own_idx) // 128))
    K += K % 2  # keep chunk counts even (own pairs + dealt pairs)
    in_maps = []
    for m in range(N_CORES):
        own_idx = own_sets[m]
        mask = np.zeros(B, bool)
        mask[own_idx] = True
        non_own = np.flatnonzero(~mask)
        n_fill = K * 128 - len(own_idx)
        fill, dealt = non_own[:n_fill], non_own[n_fill:]
        own_cols = np.concatenate([own_idx, fill])
        dsort = dealt[np.argsort(sq[dealt], kind="stable")]
        deal_mat = dsort.reshape(128, NCH - K)           # [partition, chunk]
        cols = np.concatenate([own_cols, deal_mat.T.reshape(-1)])
        mylab = lab_s[m * R : (m + 1) * R]
        in_maps.append({
            "xt": np.ascontiguousarray(xt[:, cols]),
            "xls": np.ascontiguousarray(
                (-2.0 * xs[m * R : (m + 1) * R].T)).astype(np.float16),
            "ohs": (PEN * (mylab[None, :] == np.arange(C)[:, None])).astype(np.float16),
            "ohk": (lab_s[own_cols][None, :] == np.arange(C)[:, None]).astype(np.float16),
            "sqc": np.ascontiguousarray(sq[own_cols].reshape(K, 128).T),
            "shat": sq[deal_mat].mean(1, dtype=np.float64).astype(np.float32).reshape(128, 1),
            "idn": idn,
        })
    return in_maps, lab, order, sq, K


def run_cores(embeddings, labels, trace=False, **kw):
    in_maps, lab, order, sq, K = _prep_in_maps(embeddings, labels)
    nc = _get_nc(K)
    res = run_bass_kernel_spmd(nc, in_maps, list(range(N_CORES)), trace=trace, **kw)
    hn2 = np.concatenate(
        [np.asarray(r["hn2"], np.float32).T.reshape(R) for r in res.results]
    )
    hp2 = np.concatenate(
        [np.asarray(r["hpr"], np.float32).reshape(-1, R).max(0) for r in res.results]
    )
    hn = np.sqrt(np.maximum(hn2 + sq, 0.0))
    hp = np.sqrt(np.maximum(hp2 + sq - PEN, 0.0))
    pr_sorted = np.maximum(hp - hn + 1.0, 0.0)
    pr = np.empty(B, np.float32)
    pr[order] = pr_sorted
    counts = np.bincount(lab, minlength=C)
    valid = (counts[lab] >= 2) & (counts[lab] <= B - 1)
    nv = int(valid.sum())
    loss = float((pr * valid).sum() / nv) if nv > 0 else 0.0
    return np.float32(loss), res


def kernel(embeddings, labels):
    loss, _ = run_cores(embeddings, labels, trace=False)
    return loss


# revision 18
# speedup vs baseline: 1.1799x; 1.1799x over previous
"""BatchHardTripletLoss on 8 Trainium2 NeuronCores — flipped + norm-dealt.

Layout: rows label-sorted on host; each core streams its 1024 anchors
(free dim) against all B=8192 embeddings as 64 column chunks of 128
(partition dim).

  - Own chunks (K ~ 10): the chunks holding the core's own labels (plus
    fillers), processed as K/2 psum pairs. The raw -2x.x tiles are copied
    to f16 by ACT and shipped to the HOST over the otherwise-idle DMA
    queue (512KB/pair); the host adds sq_j, masks same-label/self pairs
    exactly, and computes both the hardest-pos and the own-side
    hardest-neg. No penalty matmuls, no own-side DVE work.
  - Dealt chunks (64-K): remaining columns are sorted by ||x||^2 and
    dealt so each partition holds consecutive-rank norms across all
    chunks -> sq_j is near-constant per partition. Chunk pairs share one
    4-bank psum tile; most pairs go ACT Copy + f16 same-tile fold, some
    via a strided DVE pair-min reduce straight from psum. A binary fold
    tree (f16 2x) collapses everything; the per-partition bias s_hat is
    applied via a fused STT against a +inf dummy. The last pair takes
    the strided path and merges post-collapse to keep the tail short.
  - Partition-direction hn min via PE transposes + DVE free-dim reduces;
    sqrt/relu/mean and the own/dealt combine finalize on host.
"""

import sys

import numpy as np

if "/opt/trn_rl_repo" not in sys.path:
    sys.path.insert(0, "/opt/trn_rl_repo")

from concourse import bacc, bass, mybir, tile
from concourse.bass_utils import run_bass_kernel_spmd

B = 8192
D = 128
C = 128
N_CORES = 8
R = B // N_CORES          # anchors per core
NCH = B // 128            # column chunks (64)
RT = R // 128             # anchor blocks for the tail transposes (8)
PEN = 1024.0

F16 = mybir.dt.float16
F32 = mybir.dt.float32
ALU = mybir.AluOpType
ACTF = mybir.ActivationFunctionType

_NC_CACHE = {}


def _build_nc(kown):
    npair = (NCH - kown) // 2
    nown = kown // 2
    dve_pairs = {3, 7, 11, 15, 19, 23, npair - 1}
    nc = bacc.Bacc(None, target_bir_lowering=False)

    xt_d = nc.declare_dram_parameter("xt", [128, B], F16, isOutput=False)
    xls_d = nc.declare_dram_parameter("xls", [128, R], F16, isOutput=False)
    shat_d = nc.declare_dram_parameter("shat", [128, 1], F32, isOutput=False)
    idn_d = nc.declare_dram_parameter("idn", [128, 128], F16, isOutput=False)
    hn2_d = nc.declare_dram_parameter("hn2", [128, RT], F32, isOutput=True)
    hop_d = nc.declare_dram_parameter("hop", [128, nown * 2048], F16, isOutput=True)

    with tile.TileContext(nc) as tc:
        with tc.tile_pool(name="const", bufs=1) as cp:
            XTS = [cp.tile([128, 1024], F16, name=f"xts{s}") for s in range(8)]
            XLS = cp.tile([128, R], F16)
            SHAT = cp.tile([128, 1], F32)
            IDN = cp.tile([128, 128], F16)
            ACCF = cp.tile([128, R], F16)
            DUM = cp.tile([128, R], F16)
            HN2 = cp.tile([128, RT], F32)

            nc.scalar.dma_start(XLS[:], xls_d[:])
            nc.scalar.dma_start(SHAT[:], shat_d[:])
            nc.sync.dma_start(XTS[1][:], xt_d[:, 1024:2048])
            nc.sync.dma_start(XTS[0][:], xt_d[:, 0:1024])
            nc.sync.dma_start(XTS[2][:], xt_d[:, 2048:3072])
            for s in range(3, 8):
                nc.sync.dma_start(XTS[s][:], xt_d[:, s * 1024 : (s + 1) * 1024])
            nc.sync.dma_start(IDN[:], idn_d[:])
            nc.vector.memset(DUM[:], 60000.0)

            def chunk_lhs(ch):
                return XTS[ch // 8][:, (ch % 8) * 128 : (ch % 8) * 128 + 128]

            levels = {}

            def tree_push(level, write_fn, tp):
                buf = levels.get(level)
                if buf is None:
                    nb = tp.tile([128, 2048], F16, tag=f"tr{level}")
                    write_fn(nb[:, 0:1024])
                    levels[level] = nb
                else:
                    write_fn(buf[:, 1024:2048])
                    levels[level] = None
                    tree_push(level + 1,
                              lambda dst, b=buf: nc.vector.tensor_tensor(
                                  dst, b[:, 0:1024], b[:, 1024:2048], op=ALU.min),
                              tp)

            # schedule: 2 dealt pairs warm up (only need XLS+XT1), then the
            # own pairs, then the rest; the last dealt pair is handled after
            # the tree collapse so the tail stays short.
            sched = [("d", 0), ("d", 1)]
            sched += [("o", u) for u in range(nown)]
            sched += [("d", t) for t in range(2, npair - 1)]

            with (
                tc.tile_pool(name="dpsum", bufs=2, space=bass.MemorySpace.PSUM) as pd,
                tc.tile_pool(name="opool", bufs=nown) as op,
                tc.tile_pool(name="gpool", bufs=4) as gp,
                tc.tile_pool(name="tpool", bufs=2) as tp,
            ):
                def emit_mms(ps, kind, t):
                    for j in range(2):
                        lhs = chunk_lhs((2 * t + j) if kind == "o"
                                        else kown + 2 * t + j)
                        for h in range(2):
                            sl = slice(j * 1024 + h * 512, j * 1024 + (h + 1) * 512)
                            nc.tensor.matmul(ps[:, sl], lhs,
                                             XLS[:, h * 512 : (h + 1) * 512],
                                             start=True, stop=True)

                for kind, t in sched:
                    ps = pd.tile([128, 2048], F32, tag="ps")
                    emit_mms(ps, kind, t)
                    if kind == "o":
                        T2 = op.tile([128, 2048], F16, tag="t2")
                        nc.scalar.activation(T2[:], ps[:], ACTF.Copy)
                        nc.sync.dma_start(
                            hop_d[:, t * 2048 : (t + 1) * 2048], T2[:])
                    elif t in dve_pairs:
                        tree_push(0, lambda dst, p=ps: nc.vector.tensor_reduce(
                            dst, p[:].rearrange("p (c i) -> p i c", c=2),
                            axis=mybir.AxisListType.X, op=ALU.min), tp)
                    else:
                        G = gp.tile([128, 2048], F16, tag="g")
                        nc.scalar.activation(G[:], ps[:], ACTF.Copy)
                        tree_push(0, lambda dst, g=G: nc.vector.tensor_tensor(
                            dst, g[:, 0:1024], g[:, 1024:2048], op=ALU.min), tp)

                pend = [levels[lv][:, 0:1024]
                        for lv in sorted(levels) if levels[lv] is not None]
                nx = 0
                while len(pend) > 1:
                    a = pend.pop(0)
                    b = pend.pop(0)
                    nb = tp.tile([128, 1024], F16, tag=f"trx{nx}")
                    nx += 1
                    nc.vector.tensor_tensor(nb[:], a, b, op=ALU.min)
                    pend.append(nb[:])
                DC = pend[0]
                # last dealt pair: strided reduce, merged post-collapse
                ps = pd.tile([128, 2048], F32, tag="ps")
                emit_mms(ps, "d", npair - 1)
                WL = tp.tile([128, 1024], F16, tag="wl")
                nc.vector.tensor_reduce(
                    WL[:], ps[:].rearrange("p (c i) -> p i c", c=2),
                    axis=mybir.AxisListType.X, op=ALU.min)
                FD = tp.tile([128, 1024], F16, tag="fd")
                nc.vector.tensor_tensor(FD[:], DC, WL[:], op=ALU.min)
                # ACCF = FD + s_hat (STT against a +inf dummy keeps it one op)
                nc.vector.scalar_tensor_tensor(
                    ACCF[:], FD[:], SHAT[:, 0:1], DUM[:],
                    op0=ALU.add, op1=ALU.min,
                )

            with tc.tile_pool(name="fpsum", bufs=4, space=bass.MemorySpace.PSUM) as pf:
                for t in range(RT):
                    pn = pf.tile([128, 128], F16, tag="pn")
                    nc.tensor.transpose(pn[:], ACCF[:, t * 128 : (t + 1) * 128], IDN[:])
                    nc.vector.tensor_reduce(HN2[:, t : t + 1], pn[:],
                                            axis=mybir.AxisListType.X, op=ALU.min)

            nc.sync.dma_start(hn2_d[:], HN2[:])

    nc.compile()
    return nc


def _get_nc(kown):
    if kown not in _NC_CACHE:
        _NC_CACHE[kown] = _build_nc(kown)
    return _NC_CACHE[kown]


def _prep_in_maps(embeddings, labels):
    x = np.asarray(embeddings, dtype=np.float32)
    lab = np.asarray(labels).astype(np.int64)
    order = np.argsort(lab, kind="stable")
    lab_s = lab[order]
    xs = x[order]
    xt = np.ascontiguousarray(xs.T).astype(np.float16)   # [128, B]
    sq = (xs.astype(np.float64) ** 2).sum(1).astype(np.float32)
    idn = np.eye(128, dtype=np.float16)
    own_sets = []
    K = 0
    for m in range(N_CORES):
        mylab = lab_s[m * R : (m + 1) * R]
        own_idx = np.flatnonzero((lab_s >= mylab.min()) & (lab_s <= mylab.max()))
        own_sets.append(own_idx)
        K = max(K, -(-len(own_idx) // 128))
    K += K % 2  # keep chunk counts even (own pairs + dealt pairs)
    in_maps = []
    own_cols_all = []
    for m in range(N_CORES):
        own_idx = own_sets[m]
        mask = np.zeros(B, bool)
        mask[own_idx] = True
        non_own = np.flatnonzero(~mask)
        n_fill = K * 128 - len(own_idx)
        fill, dealt = non_own[:n_fill], non_own[n_fill:]
        own_cols = np.concatenate([own_idx, fill])
        own_cols_all.append(own_cols)
        dsort = dealt[np.argsort(sq[dealt], kind="stable")]
        deal_mat = dsort.reshape(128, NCH - K)           # [partition, chunk]
        cols = np.concatenate([own_cols, deal_mat.T.reshape(-1)])
        in_maps.append({
            "xt": np.ascontiguousarray(xt[:, cols]),
            "xls": np.ascontiguousarray(
                (-2.0 * xs[m * R : (m + 1) * R].T)).astype(np.float16),
            "shat": sq[deal_mat].mean(1, dtype=np.float64).astype(np.float32).reshape(128, 1),
            "idn": idn,
        })
    return in_maps, lab, order, lab_s, sq, K, own_cols_all


def run_cores(embeddings, labels, trace=False, **kw):
    in_maps, lab, order, lab_s, sq, K, own_cols_all = _prep_in_maps(embeddings, labels)
    nc = _get_nc(K)
    res = run_bass_kernel_spmd(nc, in_maps, list(range(N_CORES)), trace=trace, **kw)
    hn2_parts, hp2_parts = [], []
    for m, r in enumerate(res.results):
        hn2_dealt = np.asarray(r["hn2"], np.float32).T.reshape(R)
        own_cols = own_cols_all[m]
        # hop rows: [128, nown*2048] -> value of column (2u+j)*128+p at
        # anchor i lives at [p, u*2048 + j*1024 + i]
        V = np.asarray(r["hop"], np.float32).reshape(128, K // 2, 2, R)
        V = V.transpose(1, 2, 0, 3).reshape(K * 128, R)   # [own_col, anchor]
        D2 = V + sq[own_cols][:, None]
        mylab = lab_s[m * R : (m + 1) * R]
        same = lab_s[own_cols][:, None] == mylab[None, :]
        selfm = own_cols[:, None] == (m * R + np.arange(R))[None, :]
        hn_own = np.where(~same, D2, np.inf).min(0)
        hp2 = np.where(same & ~selfm, D2, -np.inf).max(0)
        hn2_parts.append(np.minimum(hn2_dealt, hn_own))
        hp2_parts.append(hp2)
    hn2 = np.concatenate(hn2_parts)
    hp2 = np.concatenate(hp2_parts)
    hn = np.sqrt(np.maximum(hn2 + sq, 0.0))
    hp = np.sqrt(np.maximum(np.where(np.isfinite(hp2), hp2, -sq) + sq, 0.0))
    pr_sorted = np.maximum(hp - hn + 1.0, 0.0)
    pr = np.empty(B, np.float32)
    pr[order] = pr_sorted
    counts = np.bincount(lab, minlength=C)
    valid = (counts[lab] >= 2) & (counts[lab] <= B - 1)
    nv = int(valid.sum())
    loss = float((pr * valid).sum() / nv) if nv > 0 else 0.0
    return np.float32(loss), res


def kernel(embeddings, labels):
    loss, _ = run_cores(embeddings, labels, trace=False)
    return loss
